# revision 1
# baseline (speedup 1.0000x reference)
"""LSH similarity-matrix kernel for Trainium2 (8 NeuronCores, data-parallel over batch).

Math: reference computes, per (l, b):
    c1 = (query_embed @ r.T > 0),  c2 = (doc_embed @ r.T > 0)   in {0,1}
    ham = s1 + s2 - 2*c1@c2.T ;  sim = cos(pi/NB * ham), masked where tok==0.
With +-1 codes U = 2c-1 and S = U1 @ U2.T:  ham = (NB - S)/2, so
    sim = sin(pi/(2*NB) * S).
Masks fold into the embeddings: a zeroed embedding row projects to 0,
sign(0) = 0 gives a zero code row, so S = 0 and sin(0) = 0 — exactly the
masked output. Masked doc tokens (half of them: tok in {0,1}) are gathered
away host-side entirely; output columns scatter back as zeros. Batches are
assigned to (core, slot) sorted by active-token count so every core runs an
identically-shaped program with minimal padding per slot.

Precision: PE fp32 matmul runs at 4 cycles/row, but float32r (TF32,
11-bit mantissa) runs at 1 cycle/row for moving dims >= 256. A single
TF32 projection flips ~1.5k hash bits (sim absmax ~9e-3), so the
projection uses the exact 3-term compensated split
    proj = rh@eh + rh@el + rl@eh,   xh = tf32(x), xl = tf32(x - xh)
which carries ~22 mantissa bits per operand and reproduces fp32 signs
(0 flips on the benchmark data; verified on hardware). The code dot runs
as fp8e4m3 DoubleRow matmuls (chunk pairs give K=256 per MM at 2 MACs/
cell/cycle); +-1/0 codes and their fp32 PSUM accumulation are exact.

r is pre-scaled by 2^66 host-side so the DVE sign alternative
clamp(x, -1, 1) = max(min(x,1),-1) is exact (any |proj| > 2^-66 maps to
+-1); sign work alternates between the ACT (Sign activation) and DVE
(clamp tensor_scalar) engines by chunk parity.
"""
import os
import sys

sys.path.insert(0, "/opt/trn_rl_repo")

from contextlib import ExitStack

import numpy as np

import concourse.bass as bass
import concourse.mybir as mybir
import concourse.tile as tile
from concourse import bacc
from concourse.bass_utils import run_bass_kernel_spmd

L, BAT, A, BDOC, D, NB = 2, 32, 64, 1024, 128, 1024
CORES = 8
BPC = BAT // CORES          # batch slots per core
CH = NB // 128              # 8 bit-chunks
SCALE = float(2.0 ** 66)
PI = float(np.pi)

F32 = mybir.dt.float32
F32R = mybir.dt.float32r
BF16 = mybir.dt.bfloat16
FP8 = mybir.dt.float8e4
Alu = mybir.AluOpType
Act = mybir.ActivationFunctionType

_BUILD_CACHE: dict = {}


def _col_splits(n):
    """Split [0, n) into equal-width pieces of <=512 columns (>=256 keeps
    float32r matmuls at full rate; a matmul may not cross a PSUM bank, so
    piece i is written at PSUM column 512*i). Equal widths mean one strided
    [p, npieces, w] access pattern covers all pieces, so sign/sin run as a
    single instruction per chunk. Returns (c0, c1, p0) per piece."""
    npieces = -(-n // 512)
    w = -(-(n // npieces) // 16) * 16
    while w * npieces < n:
        w += 16
    assert w * npieces >= n and w <= 512
    return [(i * w, min((i + 1) * w, n), 512 * i) for i in range(npieces)]


def _build(pads_c: tuple, qpad: int = A, reps: int = 1):
    """Per-core SPMD program. pads_c[b]: compute width (mult of 64) of batch
    slot b; transposes/DMA use the next multiple of 128. reps > 1 re-emits
    the whole body (timing instrumentation only)."""
    pads_c = tuple(int(p) for p in pads_c)
    pads_t = pads_c
    pad_cmax = max(pads_c)
    pad_tmax = max(pads_t)
    slot_splits = [_col_splits(p) for p in pads_c]
    np_max = max(len(s) for s in slot_splits)

    nc = bacc.Bacc("TRN2", target_bir_lowering=False, debug=False)

    QE = nc.dram_tensor("qe", [D, BPC * L * qpad], F32, kind="ExternalInput").ap()
    DE = nc.dram_tensor("de", [BPC, L, D, pad_tmax], F32, kind="ExternalInput").ap()
    RT = nc.dram_tensor("rt", [D, 2 * NB], F32, kind="ExternalInput").ap()
    OUT = nc.dram_tensor("out", [BPC, L, qpad, pad_cmax], F32, kind="ExternalOutput").ap()

    with tile.TileContext(nc) as tc, ExitStack() as ctx:
        const = ctx.enter_context(tc.tile_pool(name="const", bufs=1))
        jobp = ctx.enter_context(tc.tile_pool(name="jobp", bufs=2))
        outp = ctx.enter_context(tc.tile_pool(name="outp", bufs=2))
        ps_p = ctx.enter_context(tc.tile_pool(name="ps_p", bufs=4, space="PSUM"))

        for _rep in range(reps):
            _rp = f"r{_rep}_"
            # ---- constants; rt arrives in halves so the tf32 split of r
            # (which gates every projection) starts as early as possible ----
            # rh||rl arrive pre-split (tf32-exact) from the host; the DVE
            # copies below are identity value-wise but give the f32r-producer
            # provenance the walrus verifier requires.  Chunk-pair pieces so
            # the first projection chunks unblock as early as possible.
            rt_raw = const.tile([D, 2 * NB], F32, tag="rt_raw", name=f"{_rp}rt_raw")
            rhl = const.tile([D, 2 * NB], F32R, tag="rhl", name=f"{_rp}rhl")
            QW = BPC * L * qpad
            qnat = const.tile([D, QW], F32, tag="qnat", name=f"{_rp}qnat")

            nc.sync.dma_start(out=rt_raw[:, 0:512], in_=RT[:, 0:512])
            nc.sync.dma_start(out=rt_raw[:, NB:NB + 512], in_=RT[:, NB:NB + 512])
            for h0 in (0, NB):
                nc.vector.tensor_copy(rhl[:, h0:h0 + 512],
                                      rt_raw[:, h0:h0 + 512])

            # PE pre-warm: dependency-free dummy matmuls run while the first
            # DMAs land their completion receipts, pulling the PE through its
            # cold/mid clock ramp so the real projections start at 2.4 GHz
            warm = const.tile([D, 512], BF16, tag="warm", name=f"{_rp}warm")
            nc.gpsimd.memset(warm, 0.0)
            wps = ps_p.tile([D, 512 * np_max], F32, tag="pp",
                            name=f"{_rp}wps")[:, 0:512]
            for i in range(8):
                nc.tensor.matmul(wps, warm[:, 0:128], warm,
                                 start=True, stop=True)

            def load_consts_tail():
                # issued after the first two doc jobs' DMAs: the query side
                # and chunk 4-7 weights aren't needed until stage_b(0) ends
                nc.sync.dma_start(out=qnat, in_=QE)
                nc.sync.dma_start(out=rt_raw[:, 512:NB], in_=RT[:, 512:NB])
                nc.sync.dma_start(out=rt_raw[:, NB + 512:2 * NB],
                                  in_=RT[:, NB + 512:2 * NB])
                for h0 in (512, NB + 512):
                    nc.vector.tensor_copy(rhl[:, h0:h0 + 512],
                                          rt_raw[:, h0:h0 + 512])

            U1 = const.tile([D, CH * BPC * L * qpad], FP8, tag="U1",
                            name=f"{_rp}U1")

            def query_proj():
                qh = const.tile([D, QW], F32R, tag="qh", name=f"{_rp}qh")
                nc.vector.tensor_copy(qh, qnat)
                ql = const.tile([D, QW], F32R, tag="ql", name=f"{_rp}ql")
                nc.vector.tensor_tensor(ql, qnat, qh, Alu.subtract)
                for k in range(CH):
                    rh_k = rhl[:, k * 128:(k + 1) * 128]
                    rl_k = rhl[:, NB + k * 128:NB + (k + 1) * 128]
                    qp = ps_p.tile([D, 512 * np_max], F32, tag="pp",
                                   name=f"{_rp}qp{k}")[:, 0:QW]
                    nc.tensor.matmul(qp, rh_k, qh, start=True, stop=False)
                    nc.tensor.matmul(qp, rh_k, ql, start=False, stop=False)
                    nc.tensor.matmul(qp, rl_k, qh, start=False, stop=True)
                    u1k = U1[:, k * QW:(k + 1) * QW]
                    if k % 2 == 0:
                        nc.vector.tensor_scalar(u1k, qp, 1.0, -1.0,
                                                Alu.min, Alu.max)
                    else:
                        nc.scalar.activation(u1k, qp, Act.Sign)

            # ---- doc jobs, software-pipelined emission ----
            # stage A: dma + transpose + tf32 split;  stage B: project+sign;
            # stage C: code dot + sin + dma out.  Emitting A(j+2)/B(j+1)
            # before C(j) lets the PE run transposes/projections while
            # DVE/ACT finish the previous job's signs.
            _slot_order = sorted(range(BPC), key=lambda s: -pads_c[s])
            jobs = [(b, l) for b in _slot_order for l in range(L)]
            st = [dict() for _ in jobs]

            def stage_a(j):
                b, l = jobs[j]
                pad_t = pads_t[b]
                dnat = jobp.tile([D, pad_tmax], F32, tag="dnat",
                                 name=f"{_rp}dnat{j}")[:, 0:pad_t]
                nc.sync.dma_start(out=dnat, in_=DE[b, l, :, 0:pad_t])
                eh = jobp.tile([D, pad_tmax], F32R, tag="eh",
                               name=f"{_rp}eh{j}")[:, 0:pad_t]
                nc.vector.tensor_copy(eh, dnat)
                el = jobp.tile([D, pad_tmax], F32R, tag="el",
                               name=f"{_rp}el{j}")[:, 0:pad_t]
                nc.vector.tensor_tensor(el, dnat, eh, Alu.subtract)
                st[j]["eh"], st[j]["el"] = eh, el

            def stage_b(j):
                b, l = jobs[j]
                pad_c = pads_c[b]
                splits = slot_splits[b]
                npieces = len(splits)
                eh, el = st[j]["eh"], st[j]["el"]
                U2 = jobp.tile([D, CH * pad_cmax], FP8, tag="U2",
                               name=f"{_rp}U2{j}")
                for k in range(CH):
                    rh_k = rhl[:, k * 128:(k + 1) * 128]
                    rl_k = rhl[:, NB + k * 128:NB + (k + 1) * 128]
                    pp = ps_p.tile([D, 512 * np_max], F32, tag="pp",
                                   name=f"{_rp}pp{j}_{k}")
                    # stationary-grouped order: rh first, then rl
                    for c0, c1, p0 in splits:
                        nc.tensor.matmul(pp[:, p0:p0 + c1 - c0], rh_k,
                                         eh[:, c0:c1], start=True, stop=False)
                    for c0, c1, p0 in splits:
                        nc.tensor.matmul(pp[:, p0:p0 + c1 - c0], rh_k,
                                         el[:, c0:c1], start=False, stop=False)
                    for c0, c1, p0 in splits:
                        nc.tensor.matmul(pp[:, p0:p0 + c1 - c0], rl_k,
                                         eh[:, c0:c1], start=False, stop=True)
                    w = splits[0][1] - splits[0][0]
                    if npieces == 1 or npieces * w == pad_c:
                        if npieces == 1:
                            ppv = pp[:, 0:pad_c]
                            u2v = U2[:, k * pad_c:(k + 1) * pad_c]
                        else:
                            ppv = pp[:].rearrange("p (n c) -> p n c",
                                                  c=512)[:, 0:npieces, 0:w]
                            u2v = U2[:, k * pad_c:(k + 1) * pad_c] \
                                .rearrange("p (n c) -> p n c", c=w)
                        if k % 2 == 0:
                            nc.scalar.activation(u2v, ppv, Act.Sign)
                        else:
                            nc.vector.tensor_scalar(u2v, ppv, 1.0, -1.0,
                                                    Alu.min, Alu.max)
                    else:
                        for c0, c1, p0 in splits:
                            u2p = U2[:, k * pad_c + c0:k * pad_c + c1]
                            ppp = pp[:, p0:p0 + c1 - c0]
                            if k % 2 == 0:
                                nc.scalar.activation(u2p, ppp, Act.Sign)
                            else:
                                nc.vector.tensor_scalar(u2p, ppp, 1.0, -1.0,
                                                        Alu.min, Alu.max)
                st[j]["U2"] = U2

            def stage_c(j):
                b, l = jobs[j]
                pad_c = pads_c[b]
                splits = slot_splits[b]
                npieces = len(splits)
                U2 = st[j]["U2"]
                # code dot via fp8 DoubleRow: chunk pairs (2jj, 2jj+1) fold
                # into one K=256 matmul; +-1/0 codes are exact in fp8e4m3
                S = ps_p.tile([qpad, 512 * np_max], F32, tag="pp",
                              name=f"{_rp}S{j}")
                QW_ = BPC * L * qpad
                qcol = (b * L + l) * qpad
                for c0, c1, p0 in splits:
                    w = c1 - c0
                    for jj in range(CH // 2):
                        lw = U1[:, 2 * jj * QW_:(2 * jj + 2) * QW_] \
                            .rearrange("p (o c) -> p o c", o=2)[:, :, qcol:qcol + qpad]
                        rv = U2[:, 2 * jj * pad_c:(2 * jj + 2) * pad_c] \
                            .rearrange("p (o c) -> p o c", o=2)[:, :, c0:c1]
                        nc.tensor.matmul(
                            S[:, p0:p0 + w], lw, rv,
                            start=(jj == 0), stop=(jj == CH // 2 - 1),
                            perf_mode=mybir.MatmulPerfMode.DoubleRow,
                        )
                sim = outp.tile([qpad, pad_cmax], F32, tag="sim",
                                name=f"{_rp}sim{j}")[:, 0:pad_c]
                w = splits[0][1] - splits[0][0]
                if npieces > 1 and npieces * w == pad_c:
                    sv = S[:].rearrange("p (n c) -> p n c",
                                        c=512)[:, 0:npieces, 0:w]
                    mv = sim.rearrange("p (n c) -> p n c", c=w)
                    nc.scalar.activation(mv, sv, Act.Sin, scale=PI / (2.0 * NB))
                elif npieces == 1:
                    nc.scalar.activation(sim, S[:, 0:pad_c], Act.Sin,
                                         scale=PI / (2.0 * NB))
                else:
                    for c0, c1, p0 in splits:
                        nc.scalar.activation(sim[:, c0:c1], S[:, p0:p0 + c1 - c0],
                                             Act.Sin, scale=PI / (2.0 * NB))
                nc.sync.dma_start(out=OUT[b, l, :, 0:pad_c], in_=sim)

            n = len(jobs)
            stage_a(0)
            if n > 1:
                stage_a(1)
            load_consts_tail()
            stage_b(0)
            query_proj()
            for j in range(n - 1):
                stage_c(j)
                if j + 2 < n:
                    stage_a(j + 2)
                stage_b(j + 1)
            stage_c(n - 1)

    nc.compile()
    return nc


def _stage_inputs(query_embed, doc_embed, query_tok, doc_tok, r):
    query_embed = np.ascontiguousarray(query_embed, dtype=np.float32)
    doc_embed = np.ascontiguousarray(doc_embed, dtype=np.float32)
    r = np.ascontiguousarray(r, dtype=np.float32)

    qmask = (np.asarray(query_tok) != 0)
    dmask = (np.asarray(doc_tok) != 0)

    # sort batches by active count; slot s takes ranks [s*CORES, (s+1)*CORES)
    # spread across the 8 cores, so per-slot padding is tight and identical
    # on every core (SPMD requires one shape per slot)
    counts = dmask.sum(axis=1).astype(int)
    order = np.argsort(counts, kind="stable")
    assign = np.empty((CORES, BPC), dtype=int)   # assign[c, b] = batch id
    for s in range(BPC):
        for c in range(CORES):
            assign[c, s] = order[s * CORES + c]
    pads_c = tuple(
        min(BDOC, max(64, int(-(-int(counts[assign[:, s]].max()) // 32) * 32)))
        for s in range(BPC)
    )
    pad_tmax = max(pads_c)
    pad_cmax = max(pads_c)

    # host staging; r is pre-scaled and pre-split into tf32 hi/lo halves
    # (bit-matches fp32_to_fp32r; verified against hardware)
    def _tf32(x):
        u = np.ascontiguousarray(x, np.float32).view(np.uint32).astype(np.uint64)
        u = (u + 0x07FF + ((u >> 12) & 1)) & 0xFFFFFFFFFFFFF000
        return (u & 0xFFFFFFFF).astype(np.uint32).view(np.float32)

    qe_m = query_embed * qmask[None, :, :, None].astype(np.float32)
    qidxs = [np.flatnonzero(qmask[g]) for g in range(BAT)]
    qpad = min(A, max(16, int(-(-max(len(q) for q in qidxs) // 16) * 16)))
    rts = np.ascontiguousarray(r.T * SCALE)
    rh_host = _tf32(rts)
    rl_host = _tf32(rts - rh_host)
    rt = np.ascontiguousarray(np.concatenate([rh_host, rl_host], axis=1))

    idxs = [np.flatnonzero(dmask[g]) for g in range(BAT)]
    in_maps = []
    for c in range(CORES):
        # embeddings staged pre-transposed [D, tokens]; queries compacted
        # to their active rows (masks are per-batch, shared by both layers)
        qe_c = np.zeros((D, BPC * L * qpad), dtype=np.float32)
        de_c = np.zeros((BPC, L, D, pad_tmax), dtype=np.float32)
        for b in range(BPC):
            g = assign[c, b]
            qi = qidxs[g]
            for li in range(L):
                col = (b * L + li) * qpad
                qe_c[:, col:col + len(qi)] = qe_m[li, g, qi].T
            idx = idxs[g]
            de_c[b, :, :, :len(idx)] = doc_embed[:, g, idx].transpose(0, 2, 1)
        in_maps.append({"qe": qe_c, "de": de_c, "rt": rt})

    return in_maps, assign, idxs, pads_c, qidxs, qpad


def kernel(query_embed, doc_embed, query_tok, doc_tok, r):
    in_maps, assign, idxs, pads_c, qidxs, qpad = _stage_inputs(
        query_embed, doc_embed, query_tok, doc_tok, r)

    key = (pads_c, qpad)
    if key not in _BUILD_CACHE:
        _BUILD_CACHE[key] = _build(pads_c, qpad)
    nc = _BUILD_CACHE[key]

    res = run_bass_kernel_spmd(nc, in_maps, core_ids=list(range(CORES)))

    out = np.zeros((BAT, L, A, BDOC), dtype=np.float32)
    for c in range(CORES):
        o_c = res.results[c]["out"]  # [BPC, L, qpad, pad_cmax]
        for b in range(BPC):
            g = assign[c, b]
            idx = idxs[g]
            qi = qidxs[g]
            for li in range(L):
                out[g, li][np.ix_(qi, idx)] = o_c[b, li, :len(qi), :len(idx)]
    return out



# revision 4
# speedup vs baseline: 3106.7342x; 3106.7342x over previous
"""LSH similarity-matrix kernel for Trainium2 (8 NeuronCores, data-parallel over batch).

Math: reference computes, per (l, b):
    c1 = (query_embed @ r.T > 0),  c2 = (doc_embed @ r.T > 0)   in {0,1}
    ham = s1 + s2 - 2*c1@c2.T ;  sim = cos(pi/NB * ham), masked where tok==0.
With +-1 codes U = 2c-1 and S = U1 @ U2.T:  ham = (NB - S)/2, so
    sim = sin(pi/(2*NB) * S).
Masks fold into the embeddings: a zeroed embedding row projects to 0,
clamp(0) = 0 gives a zero code row, so S = 0 and sin(0) = 0 — exactly the
masked output. Masked doc tokens (half of them: tok in {0,1}) are gathered
away host-side entirely; output columns scatter back as zeros. Batches are
assigned to (core, slot) sorted by active-token count so every core runs an
identically-shaped program with minimal padding per slot.

Precision: projections run as single tf32 (float32r) matmuls at 1 cycle/row.
tf32's 11-bit mantissa flips ~1.5k of the 71M hash bits vs the fp32
reference (sim absmax ~9e-3, rel err ~1e-4) — far inside the tolerance.
Inputs are pre-rounded to tf32 host-side and DMA'd straight into float32r
tiles, so no on-device conversion copies are needed. The code dot runs as
fp8e4m3 DoubleRow matmuls (chunk pairs give K=256 per MM at 2 MACs/cell/
cycle); +-1/0 codes and their fp32 PSUM accumulation are exact.

r is pre-scaled by 2^66 host-side so the sign alternative
clamp(x, -1, 1) = max(min(x,1),-1) is exact (any |proj| > 2^-66 maps to
+-1). The PSUM->SBUF sign drain is the second bottleneck after the PE
(GPSIMD cannot read PSUM, so only DVE and ACT can do it); chunk signs are
assigned to DVE (clamp tensor_scalar) or ACT (Sign activation) by a greedy
cost balancer that pre-charges ACT with the per-job Sin epilogue and its
act-table loads. Output is written bf16 (exact enough; halves output DMA)
and upcast on the host.
"""
import os
import sys

sys.path.insert(0, "/opt/trn_rl_repo")

from contextlib import ExitStack

import numpy as np

import concourse.bass as bass
import concourse.mybir as mybir
import concourse.tile as tile
from concourse import bacc
from concourse.bass_utils import run_bass_kernel_spmd

L, BAT, A, BDOC, D, NB = 2, 32, 64, 1024, 128, 1024
CORES = 8
BPC = BAT // CORES          # batch slots per core
CH = NB // 128              # 8 bit-chunks
SCALE = float(2.0 ** 66)
PI = float(np.pi)

F32 = mybir.dt.float32
F32R = mybir.dt.float32r
BF16 = mybir.dt.bfloat16
FP8 = mybir.dt.float8e4
Alu = mybir.AluOpType
Act = mybir.ActivationFunctionType

NWARM = 6                   # PE ramp dummy matmuls (512 cols each)

_BUILD_CACHE: dict = {}


def _col_splits(n):
    """Split [0, n) into equal-width pieces of <=512 columns (>=256 keeps
    float32r matmuls at full rate; a matmul may not cross a PSUM bank, so
    piece i is written at PSUM column 512*i). Equal widths mean one strided
    [p, npieces, w] access pattern covers all pieces, so sign/sin run as a
    single instruction per chunk. Returns (c0, c1, p0) per piece."""
    npieces = -(-n // 512)
    w = -(-(n // npieces) // 16) * 16
    while w * npieces < n:
        w += 16
    assert w * npieces >= n and w <= 512 and npieces <= 2
    return [(i * w, min((i + 1) * w, n), 512 * i) for i in range(npieces)]


def _sign_plan(pads_c, qpad):
    """Assign each (job, chunk) doc sign to 'dve' or 'act' greedily by
    modelled engine cost (ns): DVE tensor_scalar = free*1.042 + 125,
    ACT activation = free*0.833 + 143. ACT is pre-charged with the per-job
    Sin epilogue and two act-table loads; DVE with the fused query-pair
    signs. Jobs run slots in descending-pad order, L-major."""
    QW = BPC * L * qpad
    order = sorted(range(BPC), key=lambda s: -pads_c[s])
    jobs = [(b, l) for b in order for l in range(L)]
    dve = 4 * (2 * QW * 1.042 + 125.0)
    act = sum(pads_c[b] * 0.833 + 143.0 for b, _ in jobs) + 2 * 1283.0
    plan = []
    for b, _l in jobs:
        row = []
        for _k in range(CH):
            cd = pads_c[b] * 1.042 + 125.0
            ca = pads_c[b] * 0.833 + 143.0
            if dve + cd <= act + ca:
                dve += cd
                row.append("dve")
            else:
                act += ca
                row.append("act")
        plan.append(row)
    return plan


def _build(pads_c: tuple, qpad: int = A, reps: int = 1):
    """Per-core SPMD program. pads_c[b]: compute width (mult of 16) of batch
    slot b. reps > 1 re-emits the whole body (timing instrumentation only)."""
    pads_c = tuple(int(p) for p in pads_c)
    pad_cmax = max(pads_c)
    slot_splits = [_col_splits(p) for p in pads_c]
    QW = BPC * L * qpad
    sign_plan = _sign_plan(pads_c, qpad)

    nc = bacc.Bacc("TRN2", target_bir_lowering=False, debug=False)

    QE = nc.dram_tensor("qe", [D, QW], F32R, kind="ExternalInput").ap()
    DE = nc.dram_tensor("de", [BPC, L, D, pad_cmax], F32R, kind="ExternalInput").ap()
    RT = nc.dram_tensor("rt", [D, NB], F32R, kind="ExternalInput").ap()
    OUT = nc.dram_tensor("out", [BPC, L, qpad, pad_cmax], BF16, kind="ExternalOutput").ap()

    def sign_to(eng, u, pp):
        if eng == "dve":
            nc.vector.tensor_scalar(u, pp, 1.0, -1.0, Alu.min, Alu.max)
        else:
            nc.scalar.activation(u, pp, Act.Sign)

    with tile.TileContext(nc) as tc, ExitStack() as ctx:
        const = ctx.enter_context(tc.tile_pool(name="const", bufs=1))
        jobp = ctx.enter_context(tc.tile_pool(name="jobp", bufs=2))
        outp = ctx.enter_context(tc.tile_pool(name="outp", bufs=2))
        ps_p = ctx.enter_context(tc.tile_pool(name="ps_p", bufs=4, space="PSUM"))

        for _rep in range(reps):
            _rp = f"r{_rep}_"
            # ---- constants: rt arrives in pieces so the first projection
            # chunk unblocks as early as possible; everything lands directly
            # in float32r tiles (host pre-rounds to tf32) ----
            rhl = const.tile([D, NB], F32R, tag="rhl", name=f"{_rp}rhl")
            qh = const.tile([D, QW], F32R, tag="qh", name=f"{_rp}qh")
            U1 = const.tile([D, CH * QW], FP8, tag="U1", name=f"{_rp}U1")

            _slot_order = sorted(range(BPC), key=lambda s: -pads_c[s])
            jobs = [(b, l) for b in _slot_order for l in range(L)]
            st = [dict() for _ in jobs]

            def stage_a(j):
                b, l = jobs[j]
                pad = pads_c[b]
                eh = jobp.tile([D, pad_cmax], F32R, tag="eh",
                               name=f"{_rp}eh{j}")[:, 0:pad]
                nc.sync.dma_start(out=eh, in_=DE[b, l, :, 0:pad])
                st[j]["eh"] = eh

            # DMA priority order: first proj chunk weights, first doc job,
            # the rest of the weights, queries, second doc job.
            nc.sync.dma_start(out=rhl[:, 0:128], in_=RT[:, 0:128])
            stage_a(0)
            nc.sync.dma_start(out=rhl[:, 128:NB], in_=RT[:, 128:NB])
            nc.sync.dma_start(out=qh, in_=QE)
            stage_a(1)

            # PE pre-warm: dependency-free dummy matmuls pull the PE through
            # its cold/mid clock ramp while the first DMAs land, so the real
            # projections run at 2.4 GHz
            warm = const.tile([D, 512], BF16, tag="warm", name=f"{_rp}warm")
            nc.gpsimd.memset(warm, 0.0)
            wps = ps_p.tile([D, 1024], F32, tag="pp",
                            name=f"{_rp}wps")[:, 0:512]
            for i in range(NWARM):
                nc.tensor.matmul(wps, warm[:, 0:128], warm,
                                 start=True, stop=True)

            def stage_b(j):
                b, l = jobs[j]
                pad = pads_c[b]
                splits = slot_splits[b]
                npieces = len(splits)
                w = splits[0][1] - splits[0][0]
                assert npieces * w == pad
                eh = st[j]["eh"]
                U2 = jobp.tile([D, CH * pad_cmax], FP8, tag="U2",
                               name=f"{_rp}U2{j}")
                for k in range(CH):
                    rh_k = rhl[:, k * 128:(k + 1) * 128]
                    pp = ps_p.tile([D, 1024], F32, tag="pp",
                                   name=f"{_rp}pp{j}_{k}")
                    for c0, c1, p0 in splits:
                        nc.tensor.matmul(pp[:, p0:p0 + c1 - c0], rh_k,
                                         eh[:, c0:c1], start=True, stop=True)
                    if npieces == 1:
                        ppv = pp[:, 0:pad]
                        u2v = U2[:, k * pad:(k + 1) * pad]
                    else:
                        ppv = pp[:].rearrange("p (n c) -> p n c",
                                              c=512)[:, 0:npieces, 0:w]
                        u2v = U2[:, k * pad:(k + 1) * pad] \
                            .rearrange("p (n c) -> p n c", c=w)
                    sign_to(sign_plan[j][k], u2v, ppv)
                st[j]["U2"] = U2

            def query_proj():
                # chunk pairs share one PSUM tile (cols 0 and 512) so the
                # sign runs as one fused DVE op per pair
                for kk in range(CH // 2):
                    qp = ps_p.tile([D, 1024], F32, tag="pp",
                                   name=f"{_rp}qp{kk}")
                    for h in range(2):
                        k = 2 * kk + h
                        nc.tensor.matmul(qp[:, 512 * h:512 * h + QW],
                                         rhl[:, k * 128:(k + 1) * 128], qh,
                                         start=True, stop=True)
                    u1v = U1[:, 2 * kk * QW:(2 * kk + 2) * QW] \
                        .rearrange("p (n c) -> p n c", c=QW)
                    qpv = qp[:].rearrange("p (n c) -> p n c",
                                          c=512)[:, 0:2, 0:QW]
                    nc.vector.tensor_scalar(qpv_out := u1v, qpv, 1.0, -1.0,
                                            Alu.min, Alu.max)

            def stage_c(j):
                b, l = jobs[j]
                pad = pads_c[b]
                splits = slot_splits[b]
                npieces = len(splits)
                w = splits[0][1] - splits[0][0]
                U2 = st[j]["U2"]
                # code dot via fp8 DoubleRow: chunk pairs (2jj, 2jj+1) fold
                # into one K=256 matmul; +-1/0 codes are exact in fp8e4m3
                S = ps_p.tile([qpad, 1024], F32, tag="pp",
                              name=f"{_rp}S{j}")
                qcol = (b * L + l) * qpad
                for c0, c1, p0 in splits:
                    for jj in range(CH // 2):
                        lw = U1[:, 2 * jj * QW:(2 * jj + 2) * QW] \
                            .rearrange("p (o c) -> p o c", o=2)[:, :, qcol:qcol + qpad]
                        rv = U2[:, 2 * jj * pad:(2 * jj + 2) * pad] \
                            .rearrange("p (o c) -> p o c", o=2)[:, :, c0:c1]
                        nc.tensor.matmul(
                            S[:, p0:p0 + c1 - c0], lw, rv,
                            start=(jj == 0), stop=(jj == CH // 2 - 1),
                            perf_mode=mybir.MatmulPerfMode.DoubleRow,
                        )
                sim = outp.tile([qpad, pad_cmax], BF16, tag="sim",
                                name=f"{_rp}sim{j}")[:, 0:pad]
                if npieces == 1:
                    nc.scalar.activation(sim, S[:, 0:pad], Act.Sin,
                                         scale=PI / (2.0 * NB))
                else:
                    sv = S[:].rearrange("p (n c) -> p n c",
                                        c=512)[:, 0:npieces, 0:w]
                    mv = sim.rearrange("p (n c) -> p n c", c=w)
                    nc.scalar.activation(mv, sv, Act.Sin, scale=PI / (2.0 * NB))
                nc.sync.dma_start(out=OUT[b, l, :, 0:pad], in_=sim)

            n = len(jobs)
            stage_b(0)
            query_proj()
            for j in range(n - 1):
                stage_c(j)
                if j + 2 < n:
                    stage_a(j + 2)
                stage_b(j + 1)
            stage_c(n - 1)

    nc.compile()
    return nc


def _tf32(x):
    """Round-to-nearest-even fp32 -> tf32 (11-bit mantissa), bit-matching
    the PE's fp32_to_fp32r conversion."""
    u = np.ascontiguousarray(x, np.float32).view(np.uint32).astype(np.uint64)
    u = (u + 0x07FF + ((u >> 12) & 1)) & 0xFFFFFFFFFFFFF000
    return (u & 0xFFFFFFFF).astype(np.uint32).view(np.float32)


def _stage_inputs(query_embed, doc_embed, query_tok, doc_tok, r):
    query_embed = np.ascontiguousarray(query_embed, dtype=np.float32)
    doc_embed = np.ascontiguousarray(doc_embed, dtype=np.float32)
    r = np.ascontiguousarray(r, dtype=np.float32)

    qmask = (np.asarray(query_tok) != 0)
    dmask = (np.asarray(doc_tok) != 0)

    # sort batches by active count; slot s takes ranks [s*CORES, (s+1)*CORES)
    # spread across the 8 cores, so per-slot padding is tight and identical
    # on every core (SPMD requires one shape per slot)
    counts = dmask.sum(axis=1).astype(int)
    order = np.argsort(counts, kind="stable")
    assign = np.empty((CORES, BPC), dtype=int)   # assign[c, b] = batch id
    for s in range(BPC):
        for c in range(CORES):
            assign[c, s] = order[s * CORES + c]
    def _pad(n):
        # mult of 16; slots that split across two PSUM banks need halves
        # that are themselves mult of 16, so round those to mult of 32
        p = max(64, -(-n // 16) * 16)
        if p > 512:
            p = -(-n // 32) * 32
        return min(BDOC, p)

    pads_c = tuple(_pad(int(counts[assign[:, s]].max())) for s in range(BPC))
    pad_cmax = max(pads_c)

    qe_m = query_embed * qmask[None, :, :, None].astype(np.float32)
    qidxs = [np.flatnonzero(qmask[g]) for g in range(BAT)]
    qpad = min(A, max(16, int(-(-max(len(q) for q in qidxs) // 16) * 16)))
    rt = np.ascontiguousarray(_tf32(r.T * SCALE))

    idxs = [np.flatnonzero(dmask[g]) for g in range(BAT)]
    in_maps = []
    for c in range(CORES):
        # embeddings staged pre-transposed [D, tokens], pre-rounded to tf32
        # (value-exact under the f32r DMA interpretation); queries compacted
        # to their active rows (masks are per-batch, shared by both layers)
        qe_c = np.zeros((D, BPC * L * qpad), dtype=np.float32)
        de_c = np.zeros((BPC, L, D, pad_cmax), dtype=np.float32)
        for b in range(BPC):
            g = assign[c, b]
            qi = qidxs[g]
            for li in range(L):
                col = (b * L + li) * qpad
                qe_c[:, col:col + len(qi)] = qe_m[li, g, qi].T
            idx = idxs[g]
            de_c[b, :, :, :len(idx)] = doc_embed[:, g, idx].transpose(0, 2, 1)
        in_maps.append({"qe": _tf32(qe_c), "de": _tf32(de_c), "rt": rt})

    return in_maps, assign, idxs, pads_c, qidxs, qpad


def kernel(query_embed, doc_embed, query_tok, doc_tok, r):
    in_maps, assign, idxs, pads_c, qidxs, qpad = _stage_inputs(
        query_embed, doc_embed, query_tok, doc_tok, r)

    key = (pads_c, qpad)
    if key not in _BUILD_CACHE:
        _BUILD_CACHE[key] = _build(pads_c, qpad)
    nc = _BUILD_CACHE[key]

    res = run_bass_kernel_spmd(nc, in_maps, core_ids=list(range(CORES)))

    out = np.zeros((BAT, L, A, BDOC), dtype=np.float32)
    for c in range(CORES):
        o_c = np.asarray(res.results[c]["out"]).astype(np.float32)
        for b in range(BPC):
            g = assign[c, b]
            idx = idxs[g]
            qi = qidxs[g]
            for li in range(L):
                out[g, li][np.ix_(qi, idx)] = o_c[b, li, :len(qi), :len(idx)]
    return out


# revision 6
# speedup vs baseline: 3127.1500x; 1.0066x over previous
"""LSH similarity-matrix kernel for Trainium2 (8 NeuronCores, data-parallel over batch).

Math: reference computes, per (l, b):
    c1 = (query_embed @ r.T > 0),  c2 = (doc_embed @ r.T > 0)   in {0,1}
    ham = s1 + s2 - 2*c1@c2.T ;  sim = cos(pi/NB * ham), masked where tok==0.
With +-1 codes U = 2c-1 and S = U1 @ U2.T:  ham = (NB - S)/2, so
    sim = sin(pi/(2*NB) * S).
Masks fold into the embeddings: a zeroed embedding row projects to 0,
clamp(0) = 0 gives a zero code row, so S = 0 and sin(0) = 0 — exactly the
masked output. Masked doc tokens (half of them: tok in {0,1}) are gathered
away host-side entirely; output columns scatter back as zeros. Batches are
assigned to (core, slot) sorted by active-token count so every core runs an
identically-shaped program with minimal padding per slot.

Precision: projections run as single tf32 (float32r) matmuls at 1 cycle/row.
tf32's 11-bit mantissa flips ~1.5k of the 71M hash bits vs the fp32
reference (sim absmax ~9e-3, rel err ~1e-4) — far inside the tolerance.
Inputs are pre-rounded to tf32 host-side and DMA'd straight into float32r
tiles, so no on-device conversion copies are needed. The code dot runs as
fp8e4m3 DoubleRow matmuls (chunk pairs give K=256 per MM at 2 MACs/cell/
cycle); +-1/0 codes and their fp32 PSUM accumulation are exact.

r is pre-scaled by 2^66 host-side so the sign alternative
clamp(x, -1, 1) = max(min(x,1),-1) is exact (any |proj| > 2^-66 maps to
+-1). The PSUM->SBUF sign drain is the second bottleneck after the PE
(GPSIMD cannot read PSUM, so only DVE and ACT can do it); chunk signs are
assigned to DVE (clamp tensor_scalar) or ACT (Sign activation) by a greedy
cost balancer that pre-charges ACT with the per-job Sin epilogue and its
act-table loads. Output is written bf16 (exact enough; halves output DMA)
and upcast on the host.
"""
import os
import sys

sys.path.insert(0, "/opt/trn_rl_repo")

from contextlib import ExitStack

import numpy as np

import concourse.bass as bass
import concourse.mybir as mybir
import concourse.tile as tile
from concourse import bacc
from concourse.bass_utils import run_bass_kernel_spmd

L, BAT, A, BDOC, D, NB = 2, 32, 64, 1024, 128, 1024
CORES = 8
BPC = BAT // CORES          # batch slots per core
CH = NB // 128              # 8 bit-chunks
SCALE = float(2.0 ** 66)
PI = float(np.pi)

F32 = mybir.dt.float32
F32R = mybir.dt.float32r
BF16 = mybir.dt.bfloat16
FP8 = mybir.dt.float8e4
Alu = mybir.AluOpType
Act = mybir.ActivationFunctionType

NWARM = 6                   # PE ramp dummy matmuls (512 cols each)

_BUILD_CACHE: dict = {}


def _col_splits(n):
    """Split [0, n) into equal-width pieces of <=512 columns (>=256 keeps
    float32r matmuls at full rate; a matmul may not cross a PSUM bank, so
    piece i is written at PSUM column 512*i). Equal widths mean one strided
    [p, npieces, w] access pattern covers all pieces, so sign/sin run as a
    single instruction per chunk. Returns (c0, c1, p0) per piece."""
    npieces = -(-n // 512)
    w = -(-(n // npieces) // 16) * 16
    while w * npieces < n:
        w += 16
    assert w * npieces >= n and w <= 512 and npieces <= 2
    return [(i * w, min((i + 1) * w, n), 512 * i) for i in range(npieces)]


def _sign_plan(pads_c, qpad):
    """Assign each (job, chunk) doc sign to 'dve' or 'act' greedily by
    modelled engine cost (ns): DVE tensor_scalar = free*1.042 + 125,
    ACT activation = free*0.833 + 143. Costs are charged at the point in
    the job stream where the work actually runs (sin of job j-1 lands
    during job j; query-pair signs land around job 1; act-table loads are
    hoisted to the idle warm-up window) so the split is balanced in TIME,
    not just in total. Jobs run slots in descending-pad order, L-major."""
    QW = BPC * L * qpad
    order = sorted(range(BPC), key=lambda s: -pads_c[s])
    jobs = [(b, l) for b in order for l in range(L)]
    dve = 0.0
    act = 0.0
    plan = []
    for j, (b, _l) in enumerate(jobs):
        if j == 1:
            dve += 4 * (2 * QW * 1.042 + 125.0)
        if j >= 1:
            act += pads_c[jobs[j - 1][0]] * 0.833 + 143.0
        row = []
        for _k in range(CH):
            cd = pads_c[b] * 1.042 + 125.0
            ca = pads_c[b] * 0.833 + 143.0
            if dve + cd <= act + ca:
                dve += cd
                row.append("dve")
            else:
                act += ca
                row.append("act")
        plan.append(row)
    return plan


def _build(pads_c: tuple, qpad: int = A, reps: int = 1):
    """Per-core SPMD program. pads_c[b]: compute width (mult of 16) of batch
    slot b. reps > 1 re-emits the whole body (timing instrumentation only)."""
    pads_c = tuple(int(p) for p in pads_c)
    pad_cmax = max(pads_c)
    slot_splits = [_col_splits(p) for p in pads_c]
    QW = BPC * L * qpad
    sign_plan = _sign_plan(pads_c, qpad)

    nc = bacc.Bacc("TRN2", target_bir_lowering=False, debug=False)

    QE = nc.dram_tensor("qe", [D, QW], F32R, kind="ExternalInput").ap()
    DE = nc.dram_tensor("de", [BPC, L, D, pad_cmax], F32R, kind="ExternalInput").ap()
    RT = nc.dram_tensor("rt", [D, NB], F32R, kind="ExternalInput").ap()
    OUT = nc.dram_tensor("out", [BPC, L, qpad, pad_cmax], BF16, kind="ExternalOutput").ap()

    def sign_to(eng, u, pp):
        if eng == "dve":
            nc.vector.tensor_scalar(u, pp, 1.0, -1.0, Alu.min, Alu.max)
        else:
            nc.scalar.activation(u, pp, Act.Sign)

    with tile.TileContext(nc) as tc, ExitStack() as ctx:
        const = ctx.enter_context(tc.tile_pool(name="const", bufs=1))
        jobp = ctx.enter_context(tc.tile_pool(name="jobp", bufs=2))
        outp = ctx.enter_context(tc.tile_pool(name="outp", bufs=2))
        ps_p = ctx.enter_context(tc.tile_pool(name="ps_p", bufs=4, space="PSUM"))

        for _rep in range(reps):
            _rp = f"r{_rep}_"
            # ---- constants: rt arrives in pieces so the first projection
            # chunk unblocks as early as possible; everything lands directly
            # in float32r tiles (host pre-rounds to tf32) ----
            rhl = const.tile([D, NB], F32R, tag="rhl", name=f"{_rp}rhl")
            qh = const.tile([D, QW], F32R, tag="qh", name=f"{_rp}qh")
            U1 = const.tile([D, CH * QW], FP8, tag="U1", name=f"{_rp}U1")

            _slot_order = sorted(range(BPC), key=lambda s: -pads_c[s])
            jobs = [(b, l) for b in _slot_order for l in range(L)]
            st = [dict() for _ in jobs]

            def stage_a(j):
                b, l = jobs[j]
                pad = pads_c[b]
                eh = jobp.tile([D, pad_cmax], F32R, tag="eh",
                               name=f"{_rp}eh{j}")[:, 0:pad]
                nc.sync.dma_start(out=eh, in_=DE[b, l, :, 0:pad])
                st[j]["eh"] = eh

            # DMA priority order: first proj chunk weights, first doc job,
            # the rest of the weights, queries, second doc job.
            nc.sync.dma_start(out=rhl[:, 0:128], in_=RT[:, 0:128])
            stage_a(0)
            nc.sync.dma_start(out=rhl[:, 128:NB], in_=RT[:, 128:NB])
            nc.sync.dma_start(out=qh, in_=QE)
            stage_a(1)

            # PE pre-warm: dependency-free dummy matmuls pull the PE through
            # its cold/mid clock ramp while the first DMAs land, so the real
            # projections run at 2.4 GHz
            warm = const.tile([D, 512], BF16, tag="warm", name=f"{_rp}warm")
            nc.gpsimd.memset(warm, 0.0)
            # dummy Sign + Sin on the idle ACT engine so both act-table
            # loads are hoisted into the warm-up window instead of stalling
            # the pipeline at their first real use
            wact = const.tile([D, 32], BF16, tag="wact", name=f"{_rp}wact")
            nc.scalar.activation(wact[:, 0:16], warm[:, 0:16], Act.Sign)
            nc.scalar.activation(wact[:, 16:32], warm[:, 16:32], Act.Sin)
            wps = ps_p.tile([D, 1024], F32, tag="pp",
                            name=f"{_rp}wps")[:, 0:512]
            for i in range(NWARM):
                nc.tensor.matmul(wps, warm[:, 0:128], warm,
                                 start=True, stop=True)

            def stage_b(j):
                b, l = jobs[j]
                pad = pads_c[b]
                splits = slot_splits[b]
                npieces = len(splits)
                w = splits[0][1] - splits[0][0]
                assert npieces * w == pad
                eh = st[j]["eh"]
                U2 = jobp.tile([D, CH * pad_cmax], FP8, tag="U2",
                               name=f"{_rp}U2{j}")
                for k in range(CH):
                    rh_k = rhl[:, k * 128:(k + 1) * 128]
                    pp = ps_p.tile([D, 1024], F32, tag="pp",
                                   name=f"{_rp}pp{j}_{k}")
                    for c0, c1, p0 in splits:
                        nc.tensor.matmul(pp[:, p0:p0 + c1 - c0], rh_k,
                                         eh[:, c0:c1], start=True, stop=True)
                    if npieces == 1:
                        ppv = pp[:, 0:pad]
                        u2v = U2[:, k * pad:(k + 1) * pad]
                    else:
                        ppv = pp[:].rearrange("p (n c) -> p n c",
                                              c=512)[:, 0:npieces, 0:w]
                        u2v = U2[:, k * pad:(k + 1) * pad] \
                            .rearrange("p (n c) -> p n c", c=w)
                    sign_to(sign_plan[j][k], u2v, ppv)
                st[j]["U2"] = U2

            def query_proj():
                # chunk pairs share one PSUM tile (cols 0 and 512) so the
                # sign runs as one fused DVE op per pair
                for kk in range(CH // 2):
                    qp = ps_p.tile([D, 1024], F32, tag="pp",
                                   name=f"{_rp}qp{kk}")
                    for h in range(2):
                        k = 2 * kk + h
                        nc.tensor.matmul(qp[:, 512 * h:512 * h + QW],
                                         rhl[:, k * 128:(k + 1) * 128], qh,
                                         start=True, stop=True)
                    u1v = U1[:, 2 * kk * QW:(2 * kk + 2) * QW] \
                        .rearrange("p (n c) -> p n c", c=QW)
                    qpv = qp[:].rearrange("p (n c) -> p n c",
                                          c=512)[:, 0:2, 0:QW]
                    nc.vector.tensor_scalar(qpv_out := u1v, qpv, 1.0, -1.0,
                                            Alu.min, Alu.max)

            def stage_c(j):
                b, l = jobs[j]
                pad = pads_c[b]
                splits = slot_splits[b]
                npieces = len(splits)
                w = splits[0][1] - splits[0][0]
                U2 = st[j]["U2"]
                # code dot via fp8 DoubleRow: chunk pairs (2jj, 2jj+1) fold
                # into one K=256 matmul; +-1/0 codes are exact in fp8e4m3
                S = ps_p.tile([qpad, 1024], F32, tag="pp",
                              name=f"{_rp}S{j}")
                qcol = (b * L + l) * qpad
                for c0, c1, p0 in splits:
                    for jj in range(CH // 2):
                        lw = U1[:, 2 * jj * QW:(2 * jj + 2) * QW] \
                            .rearrange("p (o c) -> p o c", o=2)[:, :, qcol:qcol + qpad]
                        rv = U2[:, 2 * jj * pad:(2 * jj + 2) * pad] \
                            .rearrange("p (o c) -> p o c", o=2)[:, :, c0:c1]
                        nc.tensor.matmul(
                            S[:, p0:p0 + c1 - c0], lw, rv,
                            start=(jj == 0), stop=(jj == CH // 2 - 1),
                            perf_mode=mybir.MatmulPerfMode.DoubleRow,
                        )
                sim = outp.tile([qpad, pad_cmax], BF16, tag="sim",
                                name=f"{_rp}sim{j}")[:, 0:pad]
                if npieces == 1:
                    nc.scalar.activation(sim, S[:, 0:pad], Act.Sin,
                                         scale=PI / (2.0 * NB))
                else:
                    sv = S[:].rearrange("p (n c) -> p n c",
                                        c=512)[:, 0:npieces, 0:w]
                    mv = sim.rearrange("p (n c) -> p n c", c=w)
                    nc.scalar.activation(mv, sv, Act.Sin, scale=PI / (2.0 * NB))
                nc.sync.dma_start(out=OUT[b, l, :, 0:pad], in_=sim)

            n = len(jobs)
            stage_b(0)
            query_proj()
            for j in range(n - 1):
                stage_c(j)
                if j + 2 < n:
                    stage_a(j + 2)
                stage_b(j + 1)
            stage_c(n - 1)

    nc.compile()
    return nc


def _tf32(x):
    """Round-to-nearest-even fp32 -> tf32 (11-bit mantissa), bit-matching
    the PE's fp32_to_fp32r conversion."""
    u = np.ascontiguousarray(x, np.float32).view(np.uint32).astype(np.uint64)
    u = (u + 0x07FF + ((u >> 12) & 1)) & 0xFFFFFFFFFFFFF000
    return (u & 0xFFFFFFFF).astype(np.uint32).view(np.float32)


def _stage_inputs(query_embed, doc_embed, query_tok, doc_tok, r):
    query_embed = np.ascontiguousarray(query_embed, dtype=np.float32)
    doc_embed = np.ascontiguousarray(doc_embed, dtype=np.float32)
    r = np.ascontiguousarray(r, dtype=np.float32)

    qmask = (np.asarray(query_tok) != 0)
    dmask = (np.asarray(doc_tok) != 0)

    # sort batches by active count; slot s takes ranks [s*CORES, (s+1)*CORES)
    # spread across the 8 cores, so per-slot padding is tight and identical
    # on every core (SPMD requires one shape per slot)
    counts = dmask.sum(axis=1).astype(int)
    order = np.argsort(counts, kind="stable")
    assign = np.empty((CORES, BPC), dtype=int)   # assign[c, b] = batch id
    for s in range(BPC):
        for c in range(CORES):
            assign[c, s] = order[s * CORES + c]
    def _pad(n):
        # mult of 16; slots that split across two PSUM banks need halves
        # that are themselves mult of 16, so round those to mult of 32
        p = max(64, -(-n // 16) * 16)
        if p > 512:
            p = -(-n // 32) * 32
        return min(BDOC, p)

    pads_c = tuple(_pad(int(counts[assign[:, s]].max())) for s in range(BPC))
    pad_cmax = max(pads_c)

    qe_m = query_embed * qmask[None, :, :, None].astype(np.float32)
    qidxs = [np.flatnonzero(qmask[g]) for g in range(BAT)]
    qpad = min(A, max(16, int(-(-max(len(q) for q in qidxs) // 16) * 16)))
    rt = np.ascontiguousarray(_tf32(r.T * SCALE))

    idxs = [np.flatnonzero(dmask[g]) for g in range(BAT)]
    in_maps = []
    for c in range(CORES):
        # embeddings staged pre-transposed [D, tokens], pre-rounded to tf32
        # (value-exact under the f32r DMA interpretation); queries compacted
        # to their active rows (masks are per-batch, shared by both layers)
        qe_c = np.zeros((D, BPC * L * qpad), dtype=np.float32)
        de_c = np.zeros((BPC, L, D, pad_cmax), dtype=np.float32)
        for b in range(BPC):
            g = assign[c, b]
            qi = qidxs[g]
            for li in range(L):
                col = (b * L + li) * qpad
                qe_c[:, col:col + len(qi)] = qe_m[li, g, qi].T
            idx = idxs[g]
            de_c[b, :, :, :len(idx)] = doc_embed[:, g, idx].transpose(0, 2, 1)
        in_maps.append({"qe": _tf32(qe_c), "de": _tf32(de_c), "rt": rt})

    return in_maps, assign, idxs, pads_c, qidxs, qpad


def kernel(query_embed, doc_embed, query_tok, doc_tok, r):
    in_maps, assign, idxs, pads_c, qidxs, qpad = _stage_inputs(
        query_embed, doc_embed, query_tok, doc_tok, r)

    key = (pads_c, qpad)
    if key not in _BUILD_CACHE:
        _BUILD_CACHE[key] = _build(pads_c, qpad)
    nc = _BUILD_CACHE[key]

    res = run_bass_kernel_spmd(nc, in_maps, core_ids=list(range(CORES)))

    out = np.zeros((BAT, L, A, BDOC), dtype=np.float32)
    for c in range(CORES):
        o_c = np.asarray(res.results[c]["out"]).astype(np.float32)
        for b in range(BPC):
            g = assign[c, b]
            idx = idxs[g]
            qi = qidxs[g]
            for li in range(L):
                out[g, li][np.ix_(qi, idx)] = o_c[b, li, :len(qi), :len(idx)]
    return out


# revision 13
# speedup vs baseline: 3236.8099x; 1.0351x over previous
"""LSH similarity-matrix kernel for Trainium2 (8 NeuronCores, data-parallel over batch).

Math: reference computes, per (l, b):
    c1 = (query_embed @ r.T > 0),  c2 = (doc_embed @ r.T > 0)   in {0,1}
    ham = s1 + s2 - 2*c1@c2.T ;  sim = cos(pi/NB * ham), masked where tok==0.
With +-1 codes U = 2c-1 and S = U1 @ U2.T:  ham = (NB - S)/2, so
    sim = sin(pi/(2*NB) * S).
Masks fold into the embeddings: a zeroed embedding row projects to 0,
clamp(0) = 0 gives a zero code row, so S = 0 and sin(0) = 0 — exactly the
masked output. Masked doc tokens (half of them: tok in {0,1}) are gathered
away host-side entirely; output columns scatter back as zeros. Batches are
assigned to (core, slot) sorted by active-token count so every core runs an
identically-shaped program with minimal padding per slot.

Precision: projections run as single tf32 (float32r) matmuls at 1 cycle/row.
tf32's 11-bit mantissa flips ~1.5k of the 71M hash bits vs the fp32
reference (sim absmax ~9e-3, rel err ~1e-4) — far inside the tolerance.
Inputs are pre-rounded to tf32 host-side and DMA'd straight into float32r
tiles, so no on-device conversion copies are needed. The code dot runs as
fp8e4m3 DoubleRow matmuls (chunk pairs give K=256 per MM at 2 MACs/cell/
cycle); +-1/0 codes and their fp32 PSUM accumulation are exact.

r is pre-scaled by 2^66 host-side so the sign alternative
clamp(x, -1, 1) = max(min(x,1),-1) is exact (any |proj| > 2^-66 maps to
+-1). The PSUM->SBUF sign drain is the second bottleneck after the PE
(GPSIMD cannot read PSUM, so only DVE and ACT can do it); chunk signs are
assigned to DVE (clamp tensor_scalar) or ACT (Sign activation) by a greedy
cost balancer that pre-charges ACT with the per-job Sin epilogue and its
act-table loads. Output is written bf16 (exact enough; halves output DMA)
and upcast on the host.
"""
import os
import sys

sys.path.insert(0, "/opt/trn_rl_repo")

from contextlib import ExitStack

import numpy as np

import concourse.bass as bass
import concourse.mybir as mybir
import concourse.tile as tile
from concourse import bacc
from concourse.bass_utils import run_bass_kernel_spmd

L, BAT, A, BDOC, D, NB = 2, 32, 64, 1024, 128, 1024
CORES = 8
BPC = BAT // CORES          # batch slots per core
CH = NB // 128              # 8 bit-chunks
SCALE = float(2.0 ** 66)
PI = float(np.pi)

F32 = mybir.dt.float32
F32R = mybir.dt.float32r
BF16 = mybir.dt.bfloat16
FP8 = mybir.dt.float8e4
Alu = mybir.AluOpType
Act = mybir.ActivationFunctionType

NWARM = 6                   # PE ramp dummy matmuls (512 cols each)

_BUILD_CACHE: dict = {}


def _col_splits(n):
    """Split [0, n) into equal-width pieces of <=512 columns (>=256 keeps
    float32r matmuls at full rate; a matmul may not cross a PSUM bank, so
    piece i is written at PSUM column 512*i). Equal widths mean one strided
    [p, npieces, w] access pattern covers all pieces, so sign/sin run as a
    single instruction per chunk. Returns (c0, c1, p0) per piece."""
    npieces = -(-n // 512)
    w = -(-(n // npieces) // 16) * 16
    while w * npieces < n:
        w += 16
    assert w * npieces >= n and w <= 512 and npieces <= 2
    return [(i * w, min((i + 1) * w, n), 512 * i) for i in range(npieces)]


def _sign_plan(pads_c, qpad):
    """Assign each (job, chunk) doc sign to 'dve' or 'act' greedily by
    modelled engine cost (ns): DVE tensor_scalar = free*1.042 + 125,
    ACT activation = free*0.833 + 143. Costs are charged at the point in
    the job stream where the work actually runs (sin of job j-1 lands
    during job j; query-pair signs land around job 1; act-table loads are
    hoisted to the idle warm-up window) so the split is balanced in TIME,
    not just in total. Jobs run slots in descending-pad order, L-major."""
    QW = BPC * L * qpad
    order = sorted(range(BPC), key=lambda s: -pads_c[s])
    jobs = [(b, l) for b in order for l in range(L)]
    dve = 0.0
    act = 0.0
    plan = []
    for j, (b, _l) in enumerate(jobs):
        if j == 2:
            dve += 2 * (2 * QW * 1.042 + 125.0)
            act += 2 * (2 * QW * 0.833 + 143.0)
        if j >= 2:
            act += pads_c[jobs[j - 2][0]] * 0.833 + 143.0
        row = []
        for _k in range(CH):
            cd = pads_c[b] * 1.042 + 125.0
            ca = pads_c[b] * 0.833 + 143.0
            if dve + cd <= act + ca:
                dve += cd
                row.append("dve")
            else:
                act += ca
                row.append("act")
        plan.append(row)
    return plan


def _build(pads_c: tuple, qpad: int = A, reps: int = 1):
    """Per-core SPMD program. pads_c[b]: compute width (mult of 16) of batch
    slot b. reps > 1 re-emits the whole body (timing instrumentation only)."""
    pads_c = tuple(int(p) for p in pads_c)
    pad_cmax = max(pads_c)
    slot_splits = [_col_splits(p) for p in pads_c]
    QW = BPC * L * qpad
    sign_plan = _sign_plan(pads_c, qpad)

    nc = bacc.Bacc("TRN2", target_bir_lowering=False, debug=False)

    QE = nc.dram_tensor("qe", [D, QW], F32R, kind="ExternalInput").ap()
    DE = nc.dram_tensor("de", [BPC, L, D, pad_cmax], F32R, kind="ExternalInput").ap()
    RT = nc.dram_tensor("rt", [D, NB], F32R, kind="ExternalInput").ap()
    OUT = nc.dram_tensor("out", [BPC, L, qpad, pad_cmax], BF16, kind="ExternalOutput").ap()

    def sign_to(eng, u, pp):
        if eng == "dve":
            nc.vector.tensor_scalar(u, pp, 1.0, -1.0, Alu.min, Alu.max)
        else:
            nc.scalar.activation(u, pp, Act.Sign)

    with tile.TileContext(nc) as tc, ExitStack() as ctx:
        const = ctx.enter_context(tc.tile_pool(name="const", bufs=1))
        ehp = ctx.enter_context(tc.tile_pool(name="ehp", bufs=4))
        u2p = ctx.enter_context(tc.tile_pool(name="u2p", bufs=3))
        outp = ctx.enter_context(tc.tile_pool(name="outp", bufs=2))
        # 8 PSUM banks: 3 x 2-bank rotating chunk tiles + 1 x 2-bank S tile
        ps_p = ctx.enter_context(tc.tile_pool(name="ps_p", bufs=3, space="PSUM"))
        ps_s = ctx.enter_context(tc.tile_pool(name="ps_s", bufs=1, space="PSUM"))

        for _rep in range(reps):
            _rp = f"r{_rep}_"
            # ---- constants: rt arrives in pieces so the first projection
            # chunk unblocks as early as possible; everything lands directly
            # in float32r tiles (host pre-rounds to tf32) ----
            rhl = const.tile([D, NB], F32R, tag="rhl", name=f"{_rp}rhl")
            qh = const.tile([D, QW], F32R, tag="qh", name=f"{_rp}qh")
            U1 = const.tile([D, CH * QW], FP8, tag="U1", name=f"{_rp}U1")

            _slot_order = sorted(range(BPC), key=lambda s: -pads_c[s])
            jobs = [(b, l) for b in _slot_order for l in range(L)]
            st = [dict() for _ in jobs]

            def stage_a(j):
                b, l = jobs[j]
                pad = pads_c[b]
                eh = ehp.tile([D, pad_cmax], F32R, tag="eh",
                              name=f"{_rp}eh{j}")[:, 0:pad]
                nc.sync.dma_start(out=eh, in_=DE[b, l, :, 0:pad])
                st[j]["eh"] = eh

            # DMA priority order: first doc job, first proj chunk weights,
            # the rest of the weights, second doc job, queries.
            stage_a(0)
            nc.sync.dma_start(out=rhl[:, 0:128], in_=RT[:, 0:128])
            nc.sync.dma_start(out=rhl[:, 128:NB], in_=RT[:, 128:NB])
            stage_a(1)
            nc.sync.dma_start(out=qh, in_=QE)

            # PE pre-warm: dependency-free dummy matmuls pull the PE through
            # its cold/mid clock ramp while the first DMAs land, so the real
            # projections run at 2.4 GHz
            warm = const.tile([D, 512], BF16, tag="warm", name=f"{_rp}warm")
            nc.gpsimd.memset(warm, 0.0)
            # dummy Sign + Sin on the idle ACT engine so both act-table
            # loads are hoisted into the warm-up window instead of stalling
            # the pipeline at their first real use
            wact = const.tile([D, 32], BF16, tag="wact", name=f"{_rp}wact")
            nc.scalar.activation(wact[:, 0:16], warm[:, 0:16], Act.Sign)
            nc.scalar.activation(wact[:, 16:32], warm[:, 16:32], Act.Sin)
            wps = ps_p.tile([D, 1024], F32, tag="pp",
                            name=f"{_rp}wps")[:, 0:512]
            for i in range(NWARM):
                nc.tensor.matmul(wps, warm[:, 0:128], warm,
                                 start=True, stop=True)

            def stage_b(j):
                b, l = jobs[j]
                pad = pads_c[b]
                splits = slot_splits[b]
                npieces = len(splits)
                w = splits[0][1] - splits[0][0]
                assert npieces * w == pad
                eh = st[j]["eh"]
                U2 = u2p.tile([D, CH * pad_cmax], FP8, tag="U2",
                              name=f"{_rp}U2{j}")
                for k in range(CH):
                    rh_k = rhl[:, k * 128:(k + 1) * 128]
                    pp = ps_p.tile([D, 1024], F32, tag="pp",
                                   name=f"{_rp}pp{j}_{k}")
                    for c0, c1, p0 in splits:
                        nc.tensor.matmul(pp[:, p0:p0 + c1 - c0], rh_k,
                                         eh[:, c0:c1], start=True, stop=True)
                    if npieces == 1:
                        ppv = pp[:, 0:pad]
                        u2v = U2[:, k * pad:(k + 1) * pad]
                    else:
                        ppv = pp[:].rearrange("p (n c) -> p n c",
                                              c=512)[:, 0:npieces, 0:w]
                        u2v = U2[:, k * pad:(k + 1) * pad] \
                            .rearrange("p (n c) -> p n c", c=w)
                    sign_to(sign_plan[j][k], u2v, ppv)
                st[j]["U2"] = U2

            def query_proj():
                # chunk pairs share one PSUM tile (cols 0 and 512) so the
                # sign runs as one fused op per pair, alternating DVE/ACT
                # so all four signs finish before the first dot needs U1
                for kk in range(CH // 2):
                    qp = ps_p.tile([D, 1024], F32, tag="pp",
                                   name=f"{_rp}qp{kk}")
                    for h in range(2):
                        k = 2 * kk + h
                        nc.tensor.matmul(qp[:, 512 * h:512 * h + QW],
                                         rhl[:, k * 128:(k + 1) * 128], qh,
                                         start=True, stop=True)
                    u1v = U1[:, 2 * kk * QW:(2 * kk + 2) * QW] \
                        .rearrange("p (n c) -> p n c", c=QW)
                    qpv = qp[:].rearrange("p (n c) -> p n c",
                                          c=512)[:, 0:2, 0:QW]
                    sign_to("dve" if kk % 2 == 0 else "act", u1v, qpv)

            def stage_c(j):
                b, l = jobs[j]
                pad = pads_c[b]
                splits = slot_splits[b]
                npieces = len(splits)
                w = splits[0][1] - splits[0][0]
                U2 = st[j]["U2"]
                # code dot via fp8 DoubleRow: chunk pairs (2jj, 2jj+1) fold
                # into one K=256 matmul; +-1/0 codes are exact in fp8e4m3
                S = ps_s.tile([qpad, 1024], F32, tag="S",
                              name=f"{_rp}S{j}")
                qcol = (b * L + l) * qpad
                for c0, c1, p0 in splits:
                    for jj in range(CH // 2):
                        lw = U1[:, 2 * jj * QW:(2 * jj + 2) * QW] \
                            .rearrange("p (o c) -> p o c", o=2)[:, :, qcol:qcol + qpad]
                        rv = U2[:, 2 * jj * pad:(2 * jj + 2) * pad] \
                            .rearrange("p (o c) -> p o c", o=2)[:, :, c0:c1]
                        nc.tensor.matmul(
                            S[:, p0:p0 + c1 - c0], lw, rv,
                            start=(jj == 0), stop=(jj == CH // 2 - 1),
                            perf_mode=mybir.MatmulPerfMode.DoubleRow,
                        )
                sim = outp.tile([qpad, pad_cmax], BF16, tag="sim",
                                name=f"{_rp}sim{j}")[:, 0:pad]
                if npieces == 1:
                    nc.scalar.activation(sim, S[:, 0:pad], Act.Sin,
                                         scale=PI / (2.0 * NB))
                else:
                    sv = S[:].rearrange("p (n c) -> p n c",
                                        c=512)[:, 0:npieces, 0:w]
                    mv = sim.rearrange("p (n c) -> p n c", c=w)
                    nc.scalar.activation(mv, sv, Act.Sin, scale=PI / (2.0 * NB))
                nc.sync.dma_start(out=OUT[b, l, :, 0:pad], in_=sim)

            # deeper pipeline: c(j) trails b(j+1), so dots/sin/output of job
            # j overlap the projections of job j+2 and the PE never waits
            # on the sign engines at job boundaries
            n = len(jobs)
            stage_b(0)
            stage_a(2)
            stage_b(1)
            query_proj()
            stage_a(3)
            for j in range(n):
                stage_c(j)
                if j + 4 < n:
                    stage_a(j + 4)
                if j + 2 < n:
                    stage_b(j + 2)

    nc.compile()
    return nc


def _tf32(x):
    """Round-to-nearest-even fp32 -> tf32 (11-bit mantissa), bit-matching
    the PE's fp32_to_fp32r conversion."""
    u = np.ascontiguousarray(x, np.float32).view(np.uint32).astype(np.uint64)
    u = (u + 0x07FF + ((u >> 12) & 1)) & 0xFFFFFFFFFFFFF000
    return (u & 0xFFFFFFFF).astype(np.uint32).view(np.float32)


def _stage_inputs(query_embed, doc_embed, query_tok, doc_tok, r):
    query_embed = np.ascontiguousarray(query_embed, dtype=np.float32)
    doc_embed = np.ascontiguousarray(doc_embed, dtype=np.float32)
    r = np.ascontiguousarray(r, dtype=np.float32)

    qmask = (np.asarray(query_tok) != 0)
    dmask = (np.asarray(doc_tok) != 0)

    # sort batches by active count; slot s takes ranks [s*CORES, (s+1)*CORES)
    # spread across the 8 cores, so per-slot padding is tight and identical
    # on every core (SPMD requires one shape per slot)
    counts = dmask.sum(axis=1).astype(int)
    order = np.argsort(counts, kind="stable")
    assign = np.empty((CORES, BPC), dtype=int)   # assign[c, b] = batch id
    for s in range(BPC):
        for c in range(CORES):
            assign[c, s] = order[s * CORES + c]
    def _pad(n):
        # mult of 16; slots that split across two PSUM banks need halves
        # that are themselves mult of 16, so round those to mult of 32
        p = max(64, -(-n // 16) * 16)
        if p > 512:
            p = -(-n // 32) * 32
        return min(BDOC, p)

    pads_c = tuple(_pad(int(counts[assign[:, s]].max())) for s in range(BPC))
    pad_cmax = max(pads_c)

    qe_m = query_embed * qmask[None, :, :, None].astype(np.float32)
    qidxs = [np.flatnonzero(qmask[g]) for g in range(BAT)]
    qpad = min(A, max(16, int(-(-max(len(q) for q in qidxs) // 16) * 16)))
    rt = np.ascontiguousarray(_tf32(r.T * SCALE))

    idxs = [np.flatnonzero(dmask[g]) for g in range(BAT)]
    in_maps = []
    for c in range(CORES):
        # embeddings staged pre-transposed [D, tokens], pre-rounded to tf32
        # (value-exact under the f32r DMA interpretation); queries compacted
        # to their active rows (masks are per-batch, shared by both layers)
        qe_c = np.zeros((D, BPC * L * qpad), dtype=np.float32)
        de_c = np.zeros((BPC, L, D, pad_cmax), dtype=np.float32)
        for b in range(BPC):
            g = assign[c, b]
            qi = qidxs[g]
            for li in range(L):
                col = (b * L + li) * qpad
                qe_c[:, col:col + len(qi)] = qe_m[li, g, qi].T
            idx = idxs[g]
            de_c[b, :, :, :len(idx)] = doc_embed[:, g, idx].transpose(0, 2, 1)
        in_maps.append({"qe": _tf32(qe_c), "de": _tf32(de_c), "rt": rt})

    return in_maps, assign, idxs, pads_c, qidxs, qpad


def kernel(query_embed, doc_embed, query_tok, doc_tok, r):
    in_maps, assign, idxs, pads_c, qidxs, qpad = _stage_inputs(
        query_embed, doc_embed, query_tok, doc_tok, r)

    key = (pads_c, qpad)
    if key not in _BUILD_CACHE:
        _BUILD_CACHE[key] = _build(pads_c, qpad)
    nc = _BUILD_CACHE[key]

    res = run_bass_kernel_spmd(nc, in_maps, core_ids=list(range(CORES)))

    out = np.zeros((BAT, L, A, BDOC), dtype=np.float32)
    for c in range(CORES):
        o_c = np.asarray(res.results[c]["out"]).astype(np.float32)
        for b in range(BPC):
            g = assign[c, b]
            idx = idxs[g]
            qi = qidxs[g]
            for li in range(L):
                out[g, li][np.ix_(qi, idx)] = o_c[b, li, :len(qi), :len(idx)]
    return out
